# revision 5
# baseline (speedup 1.0000x reference)
"""Trainium2 Bass kernel for the EnsembleModel problem.

Full inputs in, full outputs out. Internally: data-parallel over batch
across 8 NeuronCores (64 rows each); weights/index-maps replicated.

Per-core device program:
  preds = (X @ prior_W) @ dec_W (columns pre-permuted on host into
  sorted-map order), mid branch on SBUF partitions 0-63, small branch on
  partitions 64-127 of one [128, 1+10000(+pad)] buffer with column 0 = 0.0.
  Mask values at the mapped item columns are gathered chunk-by-chunk from
  contiguous mask loads (gpsimd ap_gather) and multiplied in (DVE).
  Each 4000-wide output chunk of both [64, 100000] ratings matrices is then
  produced by one gpsimd ap_gather (inverse-map indices; unmapped columns
  hit the zero sentinel) and DMA'd straight to HBM.
  Top-24 per row via 3 rounds of DVE max / max_index / match_replace on the
  small masked-preds matrices; the host maps sorted-local indices to global
  item ids through the sorted map.
"""
import sys

sys.path.insert(0, "/opt/trn_rl_repo")

import numpy as np

import concourse.mybir as mybir
from concourse import bacc
from concourse.tile import TileContext
from concourse.bass_utils import run_bass_kernel_spmd

# problem constants
B, DIN, DLAT = 512, 64, 64
NS, NM, NI = 2000, 10000, 100000
NCORES = 8
BC = B // NCORES          # 64 batch rows per core
WC = 4000                 # output-column chunk width
NCHUNK = NI // WC         # 25
PMW = 1 + NM + 23         # preds buffer width (sentinel col + preds + pad)
IDXW = WC // 16           # wrapped big-gather idx cols used per chunk
IDXS = 256                # idx-col stride per chunk (16B-aligned starts)

F32 = mybir.dt.float32
I16 = mybir.dt.int16
U32 = mybir.dt.uint32

LAST_RESULTS = None  # for test harness introspection (exec_time_ns etc.)


def _wrap16(vec, parts):
    """Wrap a 1-D index list into the gpsimd layout: index j lives at
    partition j%16, slot j//16, replicated across the 16-partition groups."""
    assert len(vec) % 16 == 0
    w = np.asarray(vec, np.int16).reshape(-1, 16).T  # [16, L/16]
    return np.tile(w, (parts // 16, 1))              # [parts, L/16]


def _mask_gather_plan(sorted_map):
    """Per chunk: wrapped local columns of the map entries in that chunk.
    Returns (idx_tensor [64, total], per-chunk list of (col_off, n, npad, s))."""
    blocks, meta, off = [], [], 0
    for c in range(NCHUNK):
        s = int(np.searchsorted(sorted_map, c * WC))
        e = int(np.searchsorted(sorted_map, (c + 1) * WC))
        n = e - s
        if n == 0:
            meta.append(None)
            continue
        npad = -(-n // 128) * 128
        loc = np.zeros(npad, np.int16)
        loc[:n] = sorted_map[s:e] - c * WC
        blocks.append(_wrap16(loc, 64))
        meta.append((off, n, npad, s))
        off += npad // 16
    idx = (np.concatenate(blocks, axis=1) if blocks
           else np.zeros((64, 1), np.int16))
    return np.ascontiguousarray(idx), meta


def _build_module(mg_meta_m, mg_meta_s, miw_m, miw_s):
    """Build the SPMD Bass program. Structure depends only on the per-chunk
    map-entry counts (host-known constants)."""
    nc = bacc.Bacc("TRN2", target_bir_lowering=False, debug=True)

    xt_d = nc.dram_tensor("xt", [DIN, BC], F32, kind="ExternalInput")
    pw_s_d = nc.dram_tensor("pw_s", [DIN, DLAT], F32, kind="ExternalInput")
    pw_m_d = nc.dram_tensor("pw_m", [DIN, DLAT], F32, kind="ExternalInput")
    dw_s_d = nc.dram_tensor("dw_s", [DLAT, NS], F32, kind="ExternalInput")
    dw_m_d = nc.dram_tensor("dw_m", [DLAT, NM], F32, kind="ExternalInput")
    mask_d = nc.dram_tensor("mask", [BC, NI], F32, kind="ExternalInput")
    bigidx_d = nc.dram_tensor("bigidx", [128, NCHUNK * IDXS], I16,
                              kind="ExternalInput")
    mi_m_d = nc.dram_tensor("mi_m", [64, miw_m], I16, kind="ExternalInput")
    mi_s_d = nc.dram_tensor("mi_s", [64, miw_s], I16, kind="ExternalInput")

    rat_s_d = nc.dram_tensor("rat_s", [BC, NI], F32, kind="ExternalOutput")
    rat_m_d = nc.dram_tensor("rat_m", [BC, NI], F32, kind="ExternalOutput")
    tk_s_d = nc.dram_tensor("tk_s", [BC, 24], U32, kind="ExternalOutput")
    tk_m_d = nc.dram_tensor("tk_m", [BC, 24], U32, kind="ExternalOutput")

    NT = 500  # moving-dim tile for the preds matmuls

    with TileContext(nc) as tc:
        with tc.tile_pool(name="consts", bufs=1) as consts, \
             tc.tile_pool(name="decw", bufs=2) as decw, \
             tc.tile_pool(name="maskp", bufs=2) as maskp, \
             tc.tile_pool(name="outp", bufs=3) as outp, \
             tc.tile_pool(name="scratch", bufs=1) as scratch, \
             tc.tile_pool(name="psum", bufs=2, space="PSUM") as psum:

            # ---- static loads -------------------------------------------
            xt_t = consts.tile([DIN, BC], F32)
            pw_s_t = consts.tile([DIN, DLAT], F32)
            pw_m_t = consts.tile([DIN, DLAT], F32)
            bigidx_t = consts.tile([128, NCHUNK * IDXS], I16)
            mi_m_t = consts.tile([64, miw_m], I16)
            mi_s_t = consts.tile([64, miw_s], I16)
            nc.sync.dma_start(out=xt_t[:], in_=xt_d[:])
            nc.sync.dma_start(out=pw_s_t[:], in_=pw_s_d[:])
            nc.sync.dma_start(out=pw_m_t[:], in_=pw_m_d[:])
            nc.sync.dma_start(out=bigidx_t[:], in_=bigidx_d[:])
            nc.sync.dma_start(out=mi_m_t[:], in_=mi_m_d[:])
            nc.sync.dma_start(out=mi_s_t[:], in_=mi_s_d[:])

            # ---- latT = prior_W.T @ X.T  ([DLAT, BC]) -------------------
            lat_m_p = psum.tile([128, NT], F32, tag="psum")
            lat_s_p = psum.tile([128, NT], F32, tag="psum")
            nc.tensor.matmul(out=lat_m_p[0:DLAT, 0:BC], lhsT=pw_m_t[:],
                             rhs=xt_t[:], start=True, stop=True)
            nc.tensor.matmul(out=lat_s_p[0:DLAT, 0:BC], lhsT=pw_s_t[:],
                             rhs=xt_t[:], start=True, stop=True)
            lat_m_t = consts.tile([DLAT, BC], F32)
            # small-branch stationary is [0 | latT_s] so preds_s lands on
            # PSUM/SBUF partitions 64-127
            lat_s_t = consts.tile([DLAT, 2 * BC], F32)
            nc.vector.tensor_copy(out=lat_m_t[:], in_=lat_m_p[0:DLAT, 0:BC])
            nc.vector.memset(lat_s_t[:, 0:BC], 0.0)
            nc.vector.tensor_copy(out=lat_s_t[:, BC:2 * BC],
                                  in_=lat_s_p[0:DLAT, 0:BC])

            # ---- preds buffer: [128, PMW]; mid lower, small upper -------
            pm = consts.tile([128, PMW], F32)
            # zeroes the sentinel column and the unused tail regions (the
            # gather's in-AP covers the whole buffer)
            nc.vector.memset(pm[:], 0.0)

            for k in range(NM // NT):  # mid: 20 tiles
                dwt = decw.tile([DLAT, NT], F32, tag="dw")
                nc.sync.dma_start(out=dwt[:], in_=dw_m_d[:, k * NT:(k + 1) * NT])
                pt = psum.tile([128, NT], F32, tag="psum")
                nc.tensor.matmul(out=pt[0:BC, :], lhsT=lat_m_t[:], rhs=dwt[:],
                                 start=True, stop=True)
                nc.vector.tensor_copy(out=pm[0:BC, 1 + k * NT:1 + (k + 1) * NT],
                                      in_=pt[0:BC, :])
            for k in range(NS // NT):  # small: 4 tiles, upper partitions
                dwt = decw.tile([DLAT, NT], F32, tag="dw")
                nc.sync.dma_start(out=dwt[:], in_=dw_s_d[:, k * NT:(k + 1) * NT])
                pt = psum.tile([128, NT], F32, tag="psum")
                nc.tensor.matmul(out=pt[:, :], lhsT=lat_s_t[:], rhs=dwt[:],
                                 start=True, stop=True)
                nc.vector.tensor_copy(out=pm[64:128, 1 + k * NT:1 + (k + 1) * NT],
                                      in_=pt[64:128, :])

            # ---- mask gather: per chunk, pick map columns ---------------
            mg_m = scratch.tile([64, NM + 160], F32, tag="sc_m")
            mg_s = scratch.tile([128, NS + 160], F32, tag="sc_s")
            for c in range(NCHUNK):
                mt = maskp.tile([BC, WC], F32, tag="mask")
                nc.sync.dma_start(out=mt[:], in_=mask_d[:, c * WC:(c + 1) * WC])
                for meta, dst, it in ((mg_meta_m[c], mg_m, mi_m_t),
                                      (mg_meta_s[c], mg_s, mi_s_t)):
                    if meta is None:
                        continue
                    off, n, npad, s = meta
                    nc.gpsimd.ap_gather(
                        out_ap=dst[0:64, s:s + npad], in_ap=mt[:],
                        idxs_ap=it[:, off:off + npad // 16],
                        channels=64, num_elems=WC, d=1, num_idxs=npad)

            # replicate small-branch mask values to partitions 64-127
            nc.sync.dma_start(out=mg_s[64:128, 0:NS], in_=mg_s[0:64, 0:NS])

            # ---- apply mask ---------------------------------------------
            nc.vector.tensor_mul(out=pm[0:BC, 1:1 + NM],
                                 in0=pm[0:BC, 1:1 + NM], in1=mg_m[0:64, 0:NM])
            nc.vector.tensor_mul(out=pm[64:128, 1:1 + NS],
                                 in0=pm[64:128, 1:1 + NS],
                                 in1=mg_s[64:128, 0:NS])

            # ---- top-24 on the small masked matrices (DVE) --------------
            tk_m_t = consts.tile([64, 24], U32)
            tk_s_t = consts.tile([128, 24], U32)
            work_m = scratch.tile([64, NM], F32, tag="sc_m")
            work_s = scratch.tile([128, NS], F32, tag="sc_s")
            m8_m = consts.tile([64, 8], F32)
            m8_s = consts.tile([128, 8], F32)
            for r in range(3):
                src_m = pm[0:BC, 1:1 + NM] if r == 0 else work_m[0:64, :]
                nc.vector.max(out=m8_m[:], in_=src_m)
                nc.vector.max_index(out=tk_m_t[:, r * 8:(r + 1) * 8],
                                    in_max=m8_m[:], in_values=src_m)
                if r < 2:
                    nc.vector.match_replace(out=work_m[0:64, :],
                                            in_to_replace=m8_m[:],
                                            in_values=src_m, imm_value=-1e30)
                src_s = pm[64:128, 1:1 + NS] if r == 0 else work_s[64:128, :]
                nc.vector.max(out=m8_s[64:128, :], in_=src_s)
                nc.vector.max_index(out=tk_s_t[64:128, r * 8:(r + 1) * 8],
                                    in_max=m8_s[64:128, :], in_values=src_s)
                if r < 2:
                    nc.vector.match_replace(out=work_s[64:128, :],
                                            in_to_replace=m8_s[64:128, :],
                                            in_values=src_s, imm_value=-1e30)
            nc.sync.dma_start(out=tk_m_d[:], in_=tk_m_t[:])
            nc.sync.dma_start(out=tk_s_d[:], in_=tk_s_t[64:128, :])

            # ---- big gathers: scatter preds into the item space ---------
            for c in range(NCHUNK):
                ot = outp.tile([128, WC], F32, tag="out")
                nc.gpsimd.ap_gather(
                    out_ap=ot[:], in_ap=pm[:, 0:PMW],
                    idxs_ap=bigidx_t[:, c * IDXS:c * IDXS + IDXW],
                    channels=128, num_elems=PMW, d=1, num_idxs=WC)
                nc.sync.dma_start(out=rat_m_d[:, c * WC:(c + 1) * WC],
                                  in_=ot[0:64, :])
                nc.sync.dma_start(out=rat_s_d[:, c * WC:(c + 1) * WC],
                                  in_=ot[64:128, :])

    nc.compile()
    return nc


def prepare(inputs):
    """Host prep + module build + per-core input maps (shared with bench)."""
    X = np.ascontiguousarray(np.asarray(inputs["X"], np.float32))
    mask = np.ascontiguousarray(np.asarray(inputs["mask"], np.float32))
    top_map = np.asarray(inputs["top_map"]).astype(np.int64)
    mid_map = np.asarray(inputs["mid_map"]).astype(np.int64)
    small_prior_W, mid_prior_W = inputs["small_prior_W"], inputs["mid_prior_W"]
    small_dec_W, mid_dec_W = inputs["small_dec_W"], inputs["mid_dec_W"]

    # ---- host prep: sort maps, permute decoder cols, inverse maps -------
    ord_s = np.argsort(top_map, kind="stable")
    ord_m = np.argsort(mid_map, kind="stable")
    sm_s = top_map[ord_s]
    sm_m = mid_map[ord_m]
    dw_s = np.ascontiguousarray(np.asarray(small_dec_W, np.float32)[:, ord_s])
    dw_m = np.ascontiguousarray(np.asarray(mid_dec_W, np.float32)[:, ord_m])

    inv_s = np.zeros(NI, np.int16)
    inv_s[sm_s] = 1 + np.arange(NS, dtype=np.int16)
    inv_m = np.zeros(NI, np.int16)
    inv_m[sm_m] = 1 + np.arange(NM, dtype=np.int16)

    # big-gather indices: per chunk, lower 4 groups mid, upper 4 small
    bigidx = np.zeros((128, NCHUNK * IDXS), np.int16)
    for c in range(NCHUNK):
        sl = slice(c * WC, (c + 1) * WC)
        col = slice(c * IDXS, c * IDXS + IDXW)
        bigidx[0:64, col] = _wrap16(inv_m[sl], 64)
        bigidx[64:128, col] = _wrap16(inv_s[sl], 64)

    mi_m, meta_m = _mask_gather_plan(sm_m)
    mi_s, meta_s = _mask_gather_plan(sm_s)

    nc = _build_module(meta_m, meta_s, mi_m.shape[1], mi_s.shape[1])

    XT = np.ascontiguousarray(X.T)  # [DIN, B]
    shared = {
        "pw_s": np.ascontiguousarray(np.asarray(small_prior_W, np.float32)),
        "pw_m": np.ascontiguousarray(np.asarray(mid_prior_W, np.float32)),
        "dw_s": dw_s, "dw_m": dw_m,
        "bigidx": bigidx, "mi_m": mi_m, "mi_s": mi_s,
    }
    in_maps = []
    for r in range(NCORES):
        in_maps.append(dict(shared,
                            xt=np.ascontiguousarray(XT[:, r * BC:(r + 1) * BC]),
                            mask=np.ascontiguousarray(mask[r * BC:(r + 1) * BC])))
    return {"nc": nc, "in_maps": in_maps, "sm_s": sm_s, "sm_m": sm_m}


def kernel(X, small_prior_W, small_dec_W, mid_prior_W, mid_dec_W,
           top_map, mid_map, mask, k):
    global LAST_RESULTS
    k = int(k)
    assert k <= 24
    prep = prepare(dict(X=X, small_prior_W=small_prior_W,
                        small_dec_W=small_dec_W, mid_prior_W=mid_prior_W,
                        mid_dec_W=mid_dec_W, top_map=top_map, mid_map=mid_map,
                        mask=mask))

    res = run_bass_kernel_spmd(prep["nc"], prep["in_maps"], list(range(NCORES)))
    LAST_RESULTS = res

    ratings_s = np.concatenate([np.asarray(r["rat_s"]) for r in res.results])
    ratings_m = np.concatenate([np.asarray(r["rat_m"]) for r in res.results])
    tk_s = np.concatenate([np.asarray(r["tk_s"]) for r in res.results])
    tk_m = np.concatenate([np.asarray(r["tk_m"]) for r in res.results])
    topk_s = prep["sm_s"][tk_s[:, :k].astype(np.int64)].astype(np.int32)
    topk_m = prep["sm_m"][tk_m[:, :k].astype(np.int64)].astype(np.int32)
    return ratings_s, ratings_m, topk_s, topk_m


# revision 7
# speedup vs baseline: 1.0831x; 1.0831x over previous
"""Trainium2 Bass kernel for the EnsembleModel problem.

Full inputs in, full outputs out. Internally: data-parallel over batch
across 8 NeuronCores (64 rows each); weights/index-maps replicated.

Per-core device program:
  preds = (X @ prior_W) @ dec_W (columns pre-permuted on host into
  sorted-map order), mid branch on SBUF partitions 0-63, small branch on
  partitions 64-127 of one [128, 1+10000(+pad)] buffer with column 0 = 0.0.
  Mask values at the mapped item columns are gathered chunk-by-chunk from
  contiguous mask loads (gpsimd ap_gather) and multiplied in (DVE).
  Each 4000-wide output chunk of both [64, 100000] ratings matrices is then
  produced by one gpsimd ap_gather (inverse-map indices; unmapped columns
  hit the zero sentinel) and DMA'd straight to HBM.
  Top-24 per row via 3 rounds of DVE max / max_index / match_replace on the
  small masked-preds matrices; the host maps sorted-local indices to global
  item ids through the sorted map.
"""
import sys

sys.path.insert(0, "/opt/trn_rl_repo")

import numpy as np

import concourse.mybir as mybir
from concourse import bacc
from concourse.tile import TileContext
from concourse.bass_utils import run_bass_kernel_spmd

# problem constants
B, DIN, DLAT = 512, 64, 64
NS, NM, NI = 2000, 10000, 100000
NCORES = 8
BC = B // NCORES          # 64 batch rows per core
WC = 4000                 # output-column chunk width
NCHUNK = NI // WC         # 25
PMW = 1 + NM + 23         # preds buffer width (sentinel col + preds + pad)
IDXW = WC // 16           # wrapped big-gather idx cols used per chunk
IDXS = 256                # idx-col stride per chunk (16B-aligned starts)

F32 = mybir.dt.float32
I16 = mybir.dt.int16
U32 = mybir.dt.uint32

LAST_RESULTS = None  # for test harness introspection (exec_time_ns etc.)


def _wrap16(vec, parts):
    """Wrap a 1-D index list into the gpsimd layout: index j lives at
    partition j%16, slot j//16, replicated across the 16-partition groups."""
    assert len(vec) % 16 == 0
    w = np.asarray(vec, np.int16).reshape(-1, 16).T  # [16, L/16]
    return np.tile(w, (parts // 16, 1))              # [parts, L/16]


def _dma_gather_plan(sorted_map):
    """Split the padded sorted entries into dma_gather calls: each call covers
    a multiple-of-128 run of entries whose item span fits int16 local offsets.
    Returns (entries_padded, calls [(count, base_item)], idx_tensor [128, *])."""
    n = len(sorted_map)
    npad = -(-n // 128) * 128
    ent = np.concatenate([sorted_map, np.full(npad - n, sorted_map[-1],
                                              sorted_map.dtype)])
    calls, i = [], 0
    while i < npad:
        j = min(i + 1024, npad)
        while ent[j - 1] - ent[i] >= 32760:
            j -= 128
        assert j > i
        calls.append((int(j - i), int(ent[i])))
        i = j
    blocks = []
    i = 0
    for cnt, base in calls:
        blocks.append(_wrap16(ent[i:i + cnt] - base, 128))
        i += cnt
    return ent, calls, np.ascontiguousarray(np.concatenate(blocks, axis=1))


def _build_module(calls_m, calls_s):
    """Build the SPMD Bass program. calls_* = per-dma_gather-call entry counts
    (host-known constants; each call covers a 128-multiple of sorted entries)."""
    nc = bacc.Bacc("TRN2", target_bir_lowering=False, debug=True)
    npad_m = sum(n for n, _ in calls_m)
    npad_s = sum(n for n, _ in calls_s)

    xt_d = nc.dram_tensor("xt", [DIN, BC], F32, kind="ExternalInput")
    pw_s_d = nc.dram_tensor("pw_s", [DIN, DLAT], F32, kind="ExternalInput")
    pw_m_d = nc.dram_tensor("pw_m", [DIN, DLAT], F32, kind="ExternalInput")
    dw_s_d = nc.dram_tensor("dw_s", [DLAT, NS], F32, kind="ExternalInput")
    dw_m_d = nc.dram_tensor("dw_m", [DLAT, NM], F32, kind="ExternalInput")
    maskt_d = nc.dram_tensor("maskt", [NI, BC], F32, kind="ExternalInput")
    bigidx_d = nc.dram_tensor("bigidx", [128, NCHUNK * IDXS], I16,
                              kind="ExternalInput")
    dgi_m_d = nc.dram_tensor("dgi_m", [128, npad_m // 16], I16,
                             kind="ExternalInput")
    dgi_s_d = nc.dram_tensor("dgi_s", [128, npad_s // 16], I16,
                             kind="ExternalInput")
    ident_d = nc.dram_tensor("ident", [128, 128], F32, kind="ExternalInput")

    rat_s_d = nc.dram_tensor("rat_s", [BC, NI], F32, kind="ExternalOutput")
    rat_m_d = nc.dram_tensor("rat_m", [BC, NI], F32, kind="ExternalOutput")
    tk_s_d = nc.dram_tensor("tk_s", [BC, 24], U32, kind="ExternalOutput")
    tk_m_d = nc.dram_tensor("tk_m", [BC, 24], U32, kind="ExternalOutput")

    NT = 500  # moving-dim tile for the preds matmuls

    with TileContext(nc) as tc:
        with tc.tile_pool(name="consts", bufs=1) as consts, \
             tc.tile_pool(name="decw", bufs=2) as decw, \
             tc.tile_pool(name="maskp", bufs=2) as maskp, \
             tc.tile_pool(name="outp", bufs=3) as outp, \
             tc.tile_pool(name="scratch", bufs=1) as scratch, \
             tc.tile_pool(name="psum", bufs=2, space="PSUM") as psum:

            # ---- static loads -------------------------------------------
            xt_t = consts.tile([DIN, BC], F32)
            pw_s_t = consts.tile([DIN, DLAT], F32)
            pw_m_t = consts.tile([DIN, DLAT], F32)
            bigidx_t = consts.tile([128, NCHUNK * IDXS], I16)
            dgi_m_t = consts.tile([128, npad_m // 16], I16)
            dgi_s_t = consts.tile([128, npad_s // 16], I16)
            ident_t = consts.tile([128, 128], F32)
            nc.sync.dma_start(out=xt_t[:], in_=xt_d[:])
            nc.sync.dma_start(out=pw_s_t[:], in_=pw_s_d[:])
            nc.sync.dma_start(out=pw_m_t[:], in_=pw_m_d[:])
            nc.sync.dma_start(out=bigidx_t[:], in_=bigidx_d[:])
            nc.sync.dma_start(out=dgi_m_t[:], in_=dgi_m_d[:])
            nc.sync.dma_start(out=dgi_s_t[:], in_=dgi_s_d[:])
            nc.sync.dma_start(out=ident_t[:], in_=ident_d[:])

            # ---- latT = prior_W.T @ X.T  ([DLAT, BC]) -------------------
            lat_m_p = psum.tile([128, NT], F32, tag="psum")
            lat_s_p = psum.tile([128, NT], F32, tag="psum")
            nc.tensor.matmul(out=lat_m_p[0:DLAT, 0:BC], lhsT=pw_m_t[:],
                             rhs=xt_t[:], start=True, stop=True)
            nc.tensor.matmul(out=lat_s_p[0:DLAT, 0:BC], lhsT=pw_s_t[:],
                             rhs=xt_t[:], start=True, stop=True)
            lat_m_t = consts.tile([DLAT, BC], F32)
            # small-branch stationary is [0 | latT_s] so preds_s lands on
            # PSUM/SBUF partitions 64-127
            lat_s_t = consts.tile([DLAT, 2 * BC], F32)
            nc.vector.tensor_copy(out=lat_m_t[:], in_=lat_m_p[0:DLAT, 0:BC])
            nc.vector.memset(lat_s_t[:, 0:BC], 0.0)
            nc.vector.tensor_copy(out=lat_s_t[:, BC:2 * BC],
                                  in_=lat_s_p[0:DLAT, 0:BC])

            # ---- preds buffer: [128, PMW]; mid lower, small upper -------
            pm = consts.tile([128, PMW], F32)
            # zeroes the sentinel column and the unused tail regions (the
            # gather's in-AP covers the whole buffer)
            nc.vector.memset(pm[:], 0.0)

            for k in range(NM // NT):  # mid: 20 tiles
                dwt = decw.tile([DLAT, NT], F32, tag="dw")
                nc.sync.dma_start(out=dwt[:], in_=dw_m_d[:, k * NT:(k + 1) * NT])
                pt = psum.tile([128, NT], F32, tag="psum")
                nc.tensor.matmul(out=pt[0:BC, :], lhsT=lat_m_t[:], rhs=dwt[:],
                                 start=True, stop=True)
                nc.vector.tensor_copy(out=pm[0:BC, 1 + k * NT:1 + (k + 1) * NT],
                                      in_=pt[0:BC, :])
            for k in range(NS // NT):  # small: 4 tiles, upper partitions
                dwt = decw.tile([DLAT, NT], F32, tag="dw")
                nc.sync.dma_start(out=dwt[:], in_=dw_s_d[:, k * NT:(k + 1) * NT])
                pt = psum.tile([128, NT], F32, tag="psum")
                nc.tensor.matmul(out=pt[:, :], lhsT=lat_s_t[:], rhs=dwt[:],
                                 start=True, stop=True)
                nc.vector.tensor_copy(out=pm[64:128, 1 + k * NT:1 + (k + 1) * NT],
                                      in_=pt[64:128, :])

            # ---- mask values at map columns: dma_gather rows of maskT ----
            # (entry-major [128, tiles, 64]), then PE-transpose each 128-entry
            # tile back to batch-major
            mg_m = scratch.tile([64, npad_m], F32, tag="sc_m")
            mg_s = scratch.tile([128, npad_s], F32, tag="sc_s")
            mgt_m = consts.tile([128, npad_m // 128, BC], F32)
            mgt_s = consts.tile([128, npad_s // 128, BC], F32)
            for (mgt, dgi, calls) in ((mgt_m, dgi_m_t, calls_m),
                                      (mgt_s, dgi_s_t, calls_s)):
                pos = 0
                for cnt, base in calls:
                    nc.gpsimd.dma_gather(
                        out_ap=mgt[:, pos // 128:(pos + cnt) // 128, :],
                        in_ap=maskt_d[base:min(base + 32760, NI), :],
                        idxs_ap=dgi[:, pos // 16:(pos + cnt) // 16],
                        num_idxs=cnt, num_idxs_reg=cnt, elem_size=BC)
                    pos += cnt
            for (mgt, mg, ntile) in ((mgt_m, mg_m, npad_m // 128),
                                     (mgt_s, mg_s, npad_s // 128)):
                for t in range(ntile):
                    tp = psum.tile([128, NT], F32, tag="psum")
                    nc.tensor.transpose(out=tp[0:BC, 0:128],
                                        in_=mgt[:, t, :], identity=ident_t[:])
                    nc.vector.tensor_copy(out=mg[0:64, t * 128:(t + 1) * 128],
                                          in_=tp[0:BC, 0:128])

            # replicate small-branch mask values to partitions 64-127
            nc.sync.dma_start(out=mg_s[64:128, 0:NS], in_=mg_s[0:64, 0:NS])

            # ---- apply mask ---------------------------------------------
            nc.vector.tensor_mul(out=pm[0:BC, 1:1 + NM],
                                 in0=pm[0:BC, 1:1 + NM], in1=mg_m[0:64, 0:NM])
            nc.vector.tensor_mul(out=pm[64:128, 1:1 + NS],
                                 in0=pm[64:128, 1:1 + NS],
                                 in1=mg_s[64:128, 0:NS])

            # ---- top-24 on the small masked matrices (DVE) --------------
            tk_m_t = consts.tile([64, 24], U32)
            tk_s_t = consts.tile([128, 24], U32)
            work_m = scratch.tile([64, NM], F32, tag="sc_m")
            work_s = scratch.tile([128, NS], F32, tag="sc_s")
            m8_m = consts.tile([64, 8], F32)
            m8_s = consts.tile([128, 8], F32)
            for r in range(3):
                src_m = pm[0:BC, 1:1 + NM] if r == 0 else work_m[0:64, :]
                nc.vector.max(out=m8_m[:], in_=src_m)
                nc.vector.max_index(out=tk_m_t[:, r * 8:(r + 1) * 8],
                                    in_max=m8_m[:], in_values=src_m)
                if r < 2:
                    nc.vector.match_replace(out=work_m[0:64, :],
                                            in_to_replace=m8_m[:],
                                            in_values=src_m, imm_value=-1e30)
                src_s = pm[64:128, 1:1 + NS] if r == 0 else work_s[64:128, :]
                nc.vector.max(out=m8_s[64:128, :], in_=src_s)
                nc.vector.max_index(out=tk_s_t[64:128, r * 8:(r + 1) * 8],
                                    in_max=m8_s[64:128, :], in_values=src_s)
                if r < 2:
                    nc.vector.match_replace(out=work_s[64:128, :],
                                            in_to_replace=m8_s[64:128, :],
                                            in_values=src_s, imm_value=-1e30)
            nc.sync.dma_start(out=tk_m_d[:], in_=tk_m_t[:])
            nc.sync.dma_start(out=tk_s_d[:], in_=tk_s_t[64:128, :])

            # ---- big gathers: scatter preds into the item space ---------
            for c in range(NCHUNK):
                ot = outp.tile([128, WC], F32, tag="out")
                nc.gpsimd.ap_gather(
                    out_ap=ot[:], in_ap=pm[:, 0:PMW],
                    idxs_ap=bigidx_t[:, c * IDXS:c * IDXS + IDXW],
                    channels=128, num_elems=PMW, d=1, num_idxs=WC)
                nc.sync.dma_start(out=rat_m_d[:, c * WC:(c + 1) * WC],
                                  in_=ot[0:64, :])
                nc.sync.dma_start(out=rat_s_d[:, c * WC:(c + 1) * WC],
                                  in_=ot[64:128, :])

    nc.compile()
    return nc


def prepare(inputs):
    """Host prep + module build + per-core input maps (shared with bench)."""
    X = np.ascontiguousarray(np.asarray(inputs["X"], np.float32))
    mask = np.ascontiguousarray(np.asarray(inputs["mask"], np.float32))
    top_map = np.asarray(inputs["top_map"]).astype(np.int64)
    mid_map = np.asarray(inputs["mid_map"]).astype(np.int64)
    small_prior_W, mid_prior_W = inputs["small_prior_W"], inputs["mid_prior_W"]
    small_dec_W, mid_dec_W = inputs["small_dec_W"], inputs["mid_dec_W"]

    # ---- host prep: sort maps, permute decoder cols, inverse maps -------
    ord_s = np.argsort(top_map, kind="stable")
    ord_m = np.argsort(mid_map, kind="stable")
    sm_s = top_map[ord_s]
    sm_m = mid_map[ord_m]
    dw_s = np.ascontiguousarray(np.asarray(small_dec_W, np.float32)[:, ord_s])
    dw_m = np.ascontiguousarray(np.asarray(mid_dec_W, np.float32)[:, ord_m])

    inv_s = np.zeros(NI, np.int16)
    inv_s[sm_s] = 1 + np.arange(NS, dtype=np.int16)
    inv_m = np.zeros(NI, np.int16)
    inv_m[sm_m] = 1 + np.arange(NM, dtype=np.int16)

    # big-gather indices: per chunk, lower 4 groups mid, upper 4 small
    bigidx = np.zeros((128, NCHUNK * IDXS), np.int16)
    for c in range(NCHUNK):
        sl = slice(c * WC, (c + 1) * WC)
        col = slice(c * IDXS, c * IDXS + IDXW)
        bigidx[0:64, col] = _wrap16(inv_m[sl], 64)
        bigidx[64:128, col] = _wrap16(inv_s[sl], 64)

    _, calls_m, dgi_m = _dma_gather_plan(sm_m)
    _, calls_s, dgi_s = _dma_gather_plan(sm_s)

    nc = _build_module(calls_m, calls_s)

    XT = np.ascontiguousarray(X.T)  # [DIN, B]
    maskT = mask.T  # [NI, B]
    shared = {
        "pw_s": np.ascontiguousarray(np.asarray(small_prior_W, np.float32)),
        "pw_m": np.ascontiguousarray(np.asarray(mid_prior_W, np.float32)),
        "dw_s": dw_s, "dw_m": dw_m,
        "bigidx": bigidx, "dgi_m": dgi_m, "dgi_s": dgi_s,
        "ident": np.eye(128, dtype=np.float32),
    }
    in_maps = []
    for r in range(NCORES):
        in_maps.append(dict(
            shared,
            xt=np.ascontiguousarray(XT[:, r * BC:(r + 1) * BC]),
            maskt=np.ascontiguousarray(maskT[:, r * BC:(r + 1) * BC])))
    return {"nc": nc, "in_maps": in_maps, "sm_s": sm_s, "sm_m": sm_m}


def kernel(X, small_prior_W, small_dec_W, mid_prior_W, mid_dec_W,
           top_map, mid_map, mask, k):
    global LAST_RESULTS
    k = int(k)
    assert k <= 24
    prep = prepare(dict(X=X, small_prior_W=small_prior_W,
                        small_dec_W=small_dec_W, mid_prior_W=mid_prior_W,
                        mid_dec_W=mid_dec_W, top_map=top_map, mid_map=mid_map,
                        mask=mask))

    res = run_bass_kernel_spmd(prep["nc"], prep["in_maps"], list(range(NCORES)))
    LAST_RESULTS = res

    ratings_s = np.concatenate([np.asarray(r["rat_s"]) for r in res.results])
    ratings_m = np.concatenate([np.asarray(r["rat_m"]) for r in res.results])
    tk_s = np.concatenate([np.asarray(r["tk_s"]) for r in res.results])
    tk_m = np.concatenate([np.asarray(r["tk_m"]) for r in res.results])
    topk_s = prep["sm_s"][tk_s[:, :k].astype(np.int64)].astype(np.int32)
    topk_m = prep["sm_m"][tk_m[:, :k].astype(np.int64)].astype(np.int32)
    return ratings_s, ratings_m, topk_s, topk_m
